# revision 2
# baseline (speedup 1.0000x reference)
"""Trainium2 Bass kernel for nn_ControlGate (bank-selected virtual linear
projection + sigmoid gate), distributed over 8 NeuronCores.

Math (per batch element b):
    W_eff = sum_k sel_probs[b,k] * W[sel_idx[b,k]]      # (d_model, d_out)
    b_eff = sum_k sel_probs[b,k] * b[sel_idx[b,k]]      # (d_out,)
    out[b] = sigmoid(tensor[b] @ W_eff + b_eff)          # (seq, d_out)

Sharding: batch==8 maps 1:1 onto the 8 cores (data parallel). The bank
gather + probability superposition is host-side input prep (as is the
transpose); each core receives its batch's tokens and effective weight in
bf16 and computes the full (seq x d_model) @ (d_model x d_out) projection,
bias add, and sigmoid on device.

Layout: output is computed TRANSPOSED — d_out on PSUM partitions, seq on
the free axis — so the bias is a per-partition scalar that fuses into the
ACT engine's activation instruction (out = sigmoid(psum + bias)) and the
epilogue needs no DVE work at all. Each 512-token chunk is processed as
two 4-PSUM-bank halves so the ACT drain of one half overlaps the PE
matmuls of the other, keeping the PE free of chunk-boundary stalls.

PE pattern: k-outer / bank-inner round-robin accumulation (measured 0.5
ns/row sustained on hw — the zero-overhead pattern; same-bank back-to-back
accumulation and per-matmul start/stop both measured slower).
"""

import os
import sys

import numpy as np

for _p in ("/opt/trn_rl_repo", "/root/.axon_site/_ro/trn_rl_repo"):
    if _p not in sys.path and os.path.isdir(_p):
        sys.path.insert(0, _p)

import concourse.bass as bass  # noqa: E402,F401
import concourse.tile as tile  # noqa: E402
from concourse import bacc, mybir  # noqa: E402
from concourse.bass_utils import run_bass_kernel_spmd  # noqa: E402

# Problem shape (hardcoded per contract)
B, S, D = 8, 4096, 1024          # batch, seq, d_model
O = 1024                         # d_out = num_heads * prod(out_shape)
NUM_HEADS, D_HEAD = 16, 64
TOP_K = 2
N_CORES = 8

P = 128                          # SBUF partitions
KT = D // P                      # 8 contraction tiles
CH = 512                         # seq columns per chunk (= one PSUM bank)
NCH = S // CH                    # 8 chunks
M = O // P                       # 8 output partition-tiles
MH = M // 2                      # o-tiles per half (4 PSUM banks)

F32 = mybir.dt.float32
F16 = mybir.dt.float16
BF16 = mybir.dt.bfloat16

_PROGRAM = None


def _build_program(bench_reps=None, mode="full"):
    """Build + compile the single-core Bass program (same NEFF on all 8 cores).

    bench_reps: timing-only variant — big IO lives in Internal DRAM (no host
    transfer) and the body repeats bench_reps times in a device-side loop.
    """
    bench = bench_reps is not None
    big_in = {"kind": "Internal"} if bench else {"kind": "ExternalInput"}
    big_out = {"kind": "Internal"} if bench else {"kind": "ExternalOutput"}
    nc = bacc.Bacc(
        "TRN2", target_bir_lowering=False, debug=False, num_devices=N_CORES
    )
    xT = nc.dram_tensor("xT", [D, S], BF16, **big_in)
    wT = nc.dram_tensor("wT", [D, O], BF16, **big_in)
    bz = nc.dram_tensor("bz", [P, M], F32, kind="ExternalInput")
    out = nc.dram_tensor("out", [O, S], F16, **big_out)
    tok = nc.dram_tensor("tok", [1, 2], F32, kind="ExternalOutput") if bench else None

    with tile.TileContext(nc) as tc:
        from contextlib import ExitStack

        with ExitStack() as ctx:
            consts = ctx.enter_context(tc.tile_pool(name="consts", bufs=1))
            wpool = ctx.enter_context(tc.tile_pool(name="w", bufs=1))
            xpool = ctx.enter_context(tc.tile_pool(name="x", bufs=3))
            opool = ctx.enter_context(tc.tile_pool(name="o", bufs=2))
            pspool = ctx.enter_context(
                tc.tile_pool(name="ps", bufs=1, space="PSUM")
            )

            bias_t = consts.tile([P, M], F32)
            nc.sync.dma_start(bias_t[:], bz.ap())

            if bench:
                ctx.enter_context(tc.For_i(0, bench_reps, 1))

            xT_r = xT.ap().rearrange("(k p) s -> p k s", p=P)
            wT_r = wT.ap().rearrange("(k p) o -> p k o", p=P)
            out_r = out.ap().rearrange("(m p) s -> p m s", p=P)

            # Weight: k=0 slice first so the PE can start early; the rest in
            # two larger transfers. All on the scalar (ACT) ring, leaving the
            # sync (SP) ring to the token stream.
            w_t = wpool.tile([P, KT, O], BF16, tag="w")
            nc.scalar.dma_start(w_t[:, 0:1, :], wT_r[:, 0:1, :])
            nc.scalar.dma_start(w_t[:, 1:4, :], wT_r[:, 1:4, :])
            nc.scalar.dma_start(w_t[:, 4:8, :], wT_r[:, 4:8, :])

            # First token chunk split per-k so matmul (k=0) fires after 128 KB.
            xs0 = xpool.tile([P, KT, CH], BF16, tag="xs")
            for k in range(KT):
                nc.sync.dma_start(xs0[:, k : k + 1, :], xT_r[:, k : k + 1, 0:CH])

            for ss in range(NCH):
                if ss == 0:
                    xs = xs0
                else:
                    cols = slice(ss * CH, (ss + 1) * CH)
                    xs = xpool.tile([P, KT, CH], BF16, tag="xs")
                    nc.sync.dma_start(xs[:], xT_r[:, :, cols])
                ostage = opool.tile([P, M, CH], F16, tag="os")
                for h in range(2):
                    ms = range(h * MH, (h + 1) * MH)
                    pss = {
                        m: pspool.tile([P, CH], F32, name=f"ps{m}", tag=f"ps{m}")
                        for m in ms
                    }
                    for k in range(KT):
                        for m in ms:
                            nc.tensor.matmul(
                                pss[m],
                                w_t[:, k, m * P : (m + 1) * P],
                                xs[:, k, :],
                                start=(k == 0),
                                stop=(k == KT - 1),
                            )
                    for m in ms:
                        nc.scalar.activation(
                            ostage[:, m, :],
                            pss[m],
                            mybir.ActivationFunctionType.Sigmoid,
                            bias=bias_t[:, m : m + 1],
                        )
                    nc.scalar.dma_start(
                        out_r[:, h * MH : (h + 1) * MH, ss * CH : (ss + 1) * CH],
                        ostage[:, h * MH : (h + 1) * MH, :],
                    )

        if tok is not None:
            nc.sync.dma_start(tok.ap(), bz.ap()[0:1, 0:2])

    nc.compile()
    return nc


def _get_program():
    global _PROGRAM
    if _PROGRAM is None:
        _PROGRAM = _build_program()
    return _PROGRAM


def _make_in_maps(tensor, sel_idx, sel_probs, W, b):
    import ml_dtypes

    bf16 = ml_dtypes.bfloat16
    tensor = np.asarray(tensor, dtype=np.float32)
    sel_idx = np.asarray(sel_idx).astype(np.int64)
    sel_probs = np.asarray(sel_probs, dtype=np.float32)
    W = np.asarray(W, dtype=np.float32)
    b = np.asarray(b, dtype=np.float32)

    in_maps = []
    for c in range(N_CORES):
        i0, i1 = sel_idx[c]
        p0, p1 = sel_probs[c]
        w_eff = (p0 * W[i0] + p1 * W[i1]).astype(np.float32)
        b_eff = (p0 * b[i0] + p1 * b[i1]).astype(np.float32)
        in_maps.append(
            {
                "xT": np.ascontiguousarray(tensor[c].T.astype(bf16)),
                "wT": w_eff.astype(bf16),
                "bz": np.ascontiguousarray(b_eff.reshape(M, P).T),
            }
        )
    return in_maps


def _execute(in_maps, trace=False, **kwargs):
    nc = _get_program()
    return run_bass_kernel_spmd(
        nc, in_maps, core_ids=list(range(N_CORES)), trace=trace, **kwargs
    )


def kernel(tensor, sel_idx, sel_probs, W, b):
    in_maps = _make_in_maps(tensor, sel_idx, sel_probs, W, b)
    res = _execute(in_maps)
    out = np.stack(
        [res.results[c]["out"].T.astype(np.float32) for c in range(N_CORES)], axis=0
    )
    return out.reshape(B, S, NUM_HEADS, D_HEAD)
